# revision 1
# baseline (speedup 1.0000x reference)
"""HDTimeCrystalBlock kernel for 8 Trainium2 NeuronCores.

Math: out = ((x @ W_in) * mod[None]) @ W_out, where
  mod[l,h] = sum_m coupled[m] * cos(omega*(m+1)*t[l] + E[m,h])
Using cos(a+b) = cos(a)cos(b) - sin(a)sin(b):
  mod = C @ A + S @ B,  A[m,h] = coupled[m]*cos(E[m,h]),
                        B[m,h] = -coupled[m]*sin(E[m,h]),
  C[l,m] = cos(omega*(m+1)*t[l]), S[l,m] = sin(...)
so the [L,M,HD] cos tensor never materializes: mod is a K=2M matmul.

Sharding: data-parallel over the 8192 tokens (B*L), 1024 per core; weights
replicated. Activations stay transposed ([feature, token]) on device so both
matmuls consume natural-layout weights as the stationary operand; the host
does the x/y transposes. Matmuls run in float32r (full PE rate, ~2e-4 rel
err). E's cos/sin run on device with sign-based range reduction to [-pi,pi]
(ACT Sin LUT is only accurate there). Weight DMAs are chunked and ordered so
the PE starts within a few microseconds and never starves (HAM stays warm).
"""
import math

import numpy as np

B, L, D, HD, M = 4, 2048, 512, 4096, 16
NCORES = 8
T = (B * L) // NCORES          # tokens per core
QCH = 512                      # l-chunk (PSUM bank width in fp32)
NQ = T // QCH
NJ = HD // 128                 # h-tiles
NK = D // 128                  # d-tiles
NC_ = HD // 1024               # w_in column chunks
JPC = 1024 // 128              # j-tiles per w_in chunk
PI = math.pi

_cache = {}


USE_BF16 = True


def _build():
    from concourse import bacc, bass, mybir, tile

    F32 = mybir.dt.float32
    F32R = mybir.dt.float32r
    MMD = mybir.dt.bfloat16 if USE_BF16 else F32R
    AF = mybir.ActivationFunctionType
    PSUM = bass.MemorySpace.PSUM

    nc = bacc.Bacc("TRN2", target_bir_lowering=False, debug=False)

    xT_d = nc.dram_tensor("xT", [D, T], MMD, kind="ExternalInput")
    w_in_d = nc.dram_tensor("w_in", [D, HD], MMD, kind="ExternalInput")
    w_out_d = nc.dram_tensor("w_out", [HD, D], MMD, kind="ExternalInput")
    cs_d = nc.dram_tensor("cs", [128, T], MMD, kind="ExternalInput")
    fe_d = nc.dram_tensor("fe", [128, M * HD // 128], F32, kind="ExternalInput")
    cbn_d = nc.dram_tensor("cbn", [128, 1], F32, kind="ExternalInput")
    yT_d = nc.dram_tensor("yT", [D, T], F32, kind="ExternalOutput")

    FEW = M * HD // 128        # 512

    with tile.TileContext(nc) as tc:
        with (
            tc.tile_pool(name="win", bufs=1) as winp,
            tc.tile_pool(name="wout", bufs=1) as woutp,
            tc.tile_pool(name="xts", bufs=1) as xtp,
            tc.tile_pool(name="small", bufs=1) as smallp,
            tc.tile_pool(name="prep", bufs=1) as prepp,
            tc.tile_pool(name="hm", bufs=3) as hmp,
            tc.tile_pool(name="mods", bufs=3) as modsp,
            tc.tile_pool(name="yo", bufs=2) as yop,
            tc.tile_pool(name="pa", bufs=2, space=PSUM) as pap,
            tc.tile_pool(name="pb", bufs=2, space=PSUM) as pbp,
            tc.tile_pool(name="py", bufs=4, space=PSUM) as pyp,
        ):
            # ---- tiny inputs first (they gate the mod path) ----
            fe = prepp.tile([128, FEW], F32, tag="fe")
            cbn = smallp.tile([128, 1], F32, tag="cbn")
            cs = smallp.tile([128, T], MMD, tag="cs")
            nc.sync.dma_start(fe[:], fe_d[:])
            nc.sync.dma_start(cbn[:], cbn_d[:])
            nc.sync.dma_start(cs[:], cs_d[:])
            wm = smallp.tile([128, 128], MMD, tag="wm")
            nc.sync.dma_start(wm[:], w_in_d[0:128, 0:128])

            # ---- E -> A=(cb*cosE), B=(-cb*sinE), in [128,512] layout ----
            # bias-free formulation (only 0.0-bias activations + one Copy):
            #   Er   = E - 2*pi*sign(E)*step(|E|-pi)       in [-pi, pi]
            #   sinE = Sin(Er)
            #   -cosE = Sin(|Er| - pi/2)
            #   A = (-cb) * (-cosE),  B = (-cb) * sinE
            sgn = prepp.tile([128, FEW], F32, tag="sgn")
            wk = prepp.tile([128, FEW], F32, tag="wk")
            er = prepp.tile([128, FEW], F32, tag="er")
            nc.scalar.activation(sgn[:], fe[:], AF.Sign)            # sign(E)
            nc.scalar.activation(wk[:], fe[:], AF.Abs)              # |E|
            nc.vector.tensor_scalar_sub(wk[:], wk[:], PI)           # |E|-pi
            nc.scalar.activation(wk[:], wk[:], AF.Sign)             # sign(|E|-pi)
            nc.scalar.activation(wk[:], wk[:], AF.Copy, bias=PI, scale=PI)
            nc.vector.tensor_mul(wk[:], wk[:], sgn[:])              # {0,±2pi}
            nc.vector.tensor_sub(er[:], fe[:], wk[:])               # Er
            nc.scalar.activation(sgn[:], er[:], AF.Sin)             # sin(E)
            nc.scalar.activation(wk[:], er[:], AF.Abs)              # |Er|
            nc.vector.tensor_scalar_sub(wk[:], wk[:], PI / 2)       # |Er|-pi/2
            nc.scalar.activation(er[:], wk[:], AF.Sin)              # -cos(E)
            a128 = prepp.tile([128, FEW], MMD, tag="a128")
            b128 = prepp.tile([128, FEW], MMD, tag="b128")
            nc.vector.tensor_scalar_mul(a128[:], er[:], cbn[:, 0:1])   # A
            nc.vector.tensor_scalar_mul(b128[:], sgn[:], cbn[:, 0:1])  # B

            # ---- reshape A,B [128,512] -> ab rows 0:2M; rows 2M:128 are
            # zero so pb runs as a uniform K=128 matmul ----
            ab = smallp.tile([128, HD], MMD, tag="ab")
            for zp in range(2 * M, 128, 32):
                nc.gpsimd.memset(ab[zp : zp + 32, :], 0.0)
            nc.gpsimd.dma_start(ab[0:M, :], a128[:, :])
            nc.gpsimd.dma_start(ab[M : 2 * M, :], b128[:, :])

            # ---- bulk loads: few large coalesced DMAs, ordered so the
            # PE starts early and never starves ----
            # win_c[c]: [128, NK, 1024] -- all 4 d-tiles of h-chunk c
            w_in_r = w_in_d.ap().rearrange("(k p) (c h) -> c p k h", p=128, c=NC_)
            xT_r = xT_d.ap().rearrange("(k p) (q t) -> q p k t", p=128, q=NQ)
            w_out_r = w_out_d.ap().rearrange("(g jj p) i -> g p jj i", p=128, jj=JPC)

            win_c = [None] * NC_
            xts_q = [None] * NQ
            wout_g = [None] * NC_

            def load_win(c):
                t_ = winp.tile([128, NK, 1024], MMD, name=f"win{c}", tag=f"win{c}")
                nc.sync.dma_start(t_[:], w_in_r[c])
                win_c[c] = t_

            def load_xts(q):
                tx = xtp.tile([128, NK, QCH], MMD, name=f"xts{q}", tag=f"xts{q}")
                nc.sync.dma_start(tx[:], xT_r[q])
                xts_q[q] = tx

            def load_wout(g):
                tw = woutp.tile([128, JPC, D], MMD, name=f"wout{g}", tag=f"wout{g}")
                nc.sync.dma_start(tw[:], w_out_r[g])
                wout_g[g] = tw

            load_xts(0)
            load_win(0)
            load_wout(0)
            for c in range(1, NC_):
                load_win(c)
                load_wout(c)
            load_xts(1)

            # ---- PE warm-up: scratch matmuls on the first-arrived tiles
            # fill the window until `ab` is ready and bring HAM to 8/8 ----
            for w in range(16):
                pw = pap.tile([128, 128], F32, name=f"warm{w}", tag="pa")
                nc.tensor.matmul(pw[:], wm[:], wm[:], start=True, stop=True)

            # ---- fused main loop (py stage software-pipelined by one j) ----
            for q in range(NQ):
                lo, hi = q * QCH, (q + 1) * QCH
                pys = [pyp.tile([128, QCH], F32, name=f"py{q}_{j2}", tag="py")
                       for j2 in range(NK)]
                pend = None
                for j in range(NJ):
                    c, jc = j // JPC, j % JPC
                    pa = pap.tile([128, QCH], F32, tag="pa")
                    for k in range(NK):
                        nc.tensor.matmul(
                            pa[:],
                            win_c[c][:, k, 128 * jc : 128 * (jc + 1)],
                            xts_q[q][:, k, :],
                            start=(k == 0),
                            stop=(k == NK - 1),
                        )
                    pb = pbp.tile([128, QCH], F32, tag="pb")
                    nc.tensor.matmul(
                        pb[:],
                        ab[:, 128 * j : 128 * (j + 1)],
                        cs[:, lo:hi],
                        start=True,
                        stop=True,
                    )
                    msb = modsp.tile([128, QCH], F32, tag="mods")
                    nc.vector.tensor_copy(msb[:], pb[:])
                    hm = hmp.tile([128, QCH], MMD, tag="hm")
                    nc.vector.tensor_mul(hm[:], pa[:], msb[:])
                    if pend is not None:
                        phm, pj = pend
                        for j2 in range(NK):
                            nc.tensor.matmul(
                                pys[j2][:],
                                wout_g[pj // JPC][:, pj % JPC,
                                                  128 * j2 : 128 * (j2 + 1)],
                                phm[:],
                                start=(pj == 0),
                                stop=False,
                            )
                    pend = (hm, j)
                phm, pj = pend
                for j2 in range(NK):
                    nc.tensor.matmul(
                        pys[j2][:],
                        wout_g[pj // JPC][:, pj % JPC, 128 * j2 : 128 * (j2 + 1)],
                        phm[:],
                        start=(pj == 0),
                        stop=True,
                    )
                for j2 in range(NK):
                    yo = yop.tile([128, QCH], F32, tag="yo")
                    nc.scalar.copy(yo[:], pys[j2][:])
                    nc.sync.dma_start(
                        yT_d[128 * j2 : 128 * (j2 + 1), lo:hi], yo[:]
                    )

    nc.finalize()
    return nc


def _get_nc():
    if "nc" not in _cache:
        _cache["nc"] = _build()
    return _cache["nc"]


def _mmd(a):
    if USE_BF16:
        import ml_dtypes
        return np.ascontiguousarray(a.astype(ml_dtypes.bfloat16))
    return np.ascontiguousarray(a.astype(np.float32))


def _in_maps(x, input_proj, output_proj, floquet_energies, drive_weights,
             coupling_matrix):
    coupled = coupling_matrix.astype(np.float64) @ drive_weights.astype(np.float64)
    cbn = (-np.repeat(coupled, 128 // M)).astype(np.float32).reshape(128, 1)
    fe = np.ascontiguousarray(
        floquet_energies.astype(np.float32).reshape(128, M * HD // 128)
    )
    w_in = _mmd(input_proj)
    w_out = _mmd(output_proj)

    harm = np.arange(1, M + 1, dtype=np.float64)
    maps = []
    for c in range(NCORES):
        b, half = c // 2, c % 2
        t = (half * T + np.arange(T, dtype=np.float64)) / L
        ang = 2.0 * np.pi * harm[:, None] * t[None, :]
        cs_np = np.zeros((128, T), dtype=np.float64)
        cs_np[0:M] = np.cos(ang)
        cs_np[M : 2 * M] = np.sin(ang)
        cs = _mmd(cs_np)
        xT = _mmd(x[b, half * T : (half + 1) * T, :].T)
        maps.append(
            {
                "xT": xT,
                "w_in": w_in,
                "w_out": w_out,
                "cs": cs,
                "fe": fe,
                "cbn": cbn,
            }
        )
    return maps


def kernel(x, input_proj, output_proj, floquet_energies, drive_weights,
           coupling_matrix, _trace=False, _trace_kwargs=None):
    from concourse.bass_utils import run_bass_kernel_spmd

    nc = _get_nc()
    maps = _in_maps(x, input_proj, output_proj, floquet_energies,
                    drive_weights, coupling_matrix)
    kw = dict(_trace_kwargs or {})
    res = run_bass_kernel_spmd(nc, maps, list(range(NCORES)), trace=_trace, **kw)
    out = np.empty((B, L, D), dtype=np.float32)
    for c in range(NCORES):
        b, half = c // 2, c % 2
        out[b, half * T : (half + 1) * T, :] = res.results[c]["yT"].T
    if _trace:
        return out, res
    return out



# revision 4
# speedup vs baseline: 1.0087x; 1.0087x over previous
"""HDTimeCrystalBlock kernel for 8 Trainium2 NeuronCores.

Math: out = ((x @ W_in) * mod[None]) @ W_out, where
  mod[l,h] = sum_m coupled[m] * cos(omega*(m+1)*t[l] + E[m,h])
Using cos(a+b) = cos(a)cos(b) - sin(a)sin(b):
  mod = cs^T-style K=32 matmul:  mod[h,l] = sum_r ab[r,h] * cs[r,l]
  ab rows 0:16  = coupled[m]*cos(E[m,h])
  ab rows 16:32 = -coupled[m]*sin(E[m,h])
  cs rows 0:16  = cos(omega*(m+1)*t[l]),  rows 16:32 = sin(...)
ab and cs are tiny ([32,4096] / [32,T]) and are precomputed on the HOST,
so the device does no trig/prep at all - the old on-device prep chain
(act-table loads, gpsimd memsets, sin/cos range reduction) delayed the
first mod matmul to ~29us and kept the PE HAM-throttled at 1.2 GHz for
the first ~35us of the kernel.

Sharding: data-parallel over the 8192 tokens (B*L), 1024 per core; weights
replicated. Activations stay transposed ([feature, token]) on device so both
matmuls consume natural-layout weights as the stationary operand; the host
does the x/y transposes. All matmul operands bf16 (1 col/cycle at 2.4 GHz).

Startup discipline (the whole point of v2):
  - PE warm-up matmuls run on a memset scratch tile -> no DMA dependency,
    so the PE is busy from ~5.5us and HAM un-throttles to 2.4 GHz ASAP.
  - Input DMAs are split across the 3 DMA-capable queues (sync/SP,
    scalar/Act, gpsimd) and ordered in exact consumption order, so the
    main loop starts by ~10us fully warm and never starves.
Output is written bf16 (halves output traffic; adds ~0.2% worst-case
element error vs a 2e-2 budget).
"""
import math

import numpy as np

B, L, D, HD, M = 4, 2048, 512, 4096, 16
NCORES = 8
T = (B * L) // NCORES          # tokens per core
QCH = 512                      # l-chunk (PSUM bank width in fp32)
NQ = T // QCH
NJ = HD // 128                 # h-tiles
NK = D // 128                  # d-tiles
WCH = 8                        # w_in / w_out column chunks (512 cols each)
JPW = NJ // WCH                # j-tiles per chunk (4)
K2 = 2 * M                     # mod-matmul contraction (32)
NWARM = 8                      # scratch warm-up matmuls (N=512)

_cache = {}


def _build():
    from concourse import bacc, bass, mybir, tile

    F32 = mybir.dt.float32
    BF16 = mybir.dt.bfloat16
    PSUM = bass.MemorySpace.PSUM

    nc = bacc.Bacc("TRN2", target_bir_lowering=False, debug=False)

    xT_d = nc.dram_tensor("xT", [D, T], BF16, kind="ExternalInput")
    w_in_d = nc.dram_tensor("w_in", [D, HD], BF16, kind="ExternalInput")
    w_out_d = nc.dram_tensor("w_out", [HD, D], BF16, kind="ExternalInput")
    cs_d = nc.dram_tensor("cs", [K2, T], BF16, kind="ExternalInput")
    ab_d = nc.dram_tensor("ab", [K2, HD], BF16, kind="ExternalInput")
    yT_d = nc.dram_tensor("yT", [D, T], BF16, kind="ExternalOutput")

    with tile.TileContext(nc) as tc:
        with (
            tc.tile_pool(name="win", bufs=1) as winp,
            tc.tile_pool(name="wout", bufs=1) as woutp,
            tc.tile_pool(name="xts", bufs=1) as xtp,
            tc.tile_pool(name="small", bufs=1) as smallp,
            tc.tile_pool(name="hm", bufs=3) as hmp,
            tc.tile_pool(name="mods", bufs=3) as modsp,
            tc.tile_pool(name="yo", bufs=4) as yop,
            tc.tile_pool(name="pa", bufs=2, space=PSUM) as pap,
            tc.tile_pool(name="pb", bufs=2, space=PSUM) as pbp,
            tc.tile_pool(name="py", bufs=4, space=PSUM) as pyp,
        ):
            # ---- scratch warm-up tile: no DMA dependency ----
            wmt = smallp.tile([128, QCH], BF16, tag="wmt")
            nc.vector.memset(wmt[:], 0.5)

            # ---- DMA schedule: 3 queues, consumption order ----
            # sync (SP):    win chunks 0..7, then xts1
            # scalar (Act): xts0 (first!), later the yo stores
            # gpsimd:       ab, cs, wout chunks 0..7
            w_in_r = w_in_d.ap().rearrange("(k p) (c h) -> c p k h", p=128, c=WCH)
            xT_r = xT_d.ap().rearrange("(k p) (q t) -> q p k t", p=128, q=NQ)
            w_out_r = w_out_d.ap().rearrange("(g jj p) i -> g p jj i", p=128, jj=JPW)

            win_c = [None] * WCH
            wout_g = [None] * WCH
            xts_q = [None] * NQ

            def load_win(c):
                t_ = winp.tile([128, NK, 512], BF16, name=f"win{c}", tag=f"win{c}")
                nc.sync.dma_start(t_[:], w_in_r[c])
                win_c[c] = t_

            def load_xts(q, eng):
                tx = xtp.tile([128, NK, T // NQ], BF16, name=f"xts{q}", tag=f"xts{q}")
                eng.dma_start(tx[:], xT_r[q])
                xts_q[q] = tx

            def load_wout(g):
                tw = woutp.tile([128, JPW, D], BF16, name=f"wout{g}", tag=f"wout{g}")
                nc.gpsimd.dma_start(tw[:], w_out_r[g])
                wout_g[g] = tw

            ab = smallp.tile([K2, HD], BF16, tag="ab")
            cs = smallp.tile([K2, T], BF16, tag="cs")

            load_xts(0, nc.scalar)
            load_win(0)
            nc.gpsimd.dma_start(ab[:], ab_d[:])
            nc.gpsimd.dma_start(cs[:], cs_d[:])
            load_win(1)
            load_wout(0)
            load_wout(1)
            for c in range(2, WCH):
                load_win(c)
                load_wout(c)
            load_xts(1, nc.sync)

            # ---- PE warm-up on scratch: busy from ~5.5us so HAM hits
            # K=8/8 (2.4 GHz) right as the real data lands ----
            for w in range(NWARM):
                pw = pap.tile([128, QCH], F32, name=f"warm{w}", tag="pa")
                nc.tensor.matmul(pw[:], wmt[:, 0:128], wmt[:], start=True, stop=True)

            # ---- fused main loop (py stage software-pipelined by one j) ----
            for q in range(NQ):
                lo, hi = q * QCH, (q + 1) * QCH
                pys = [pyp.tile([128, QCH], F32, name=f"py{q}_{j2}", tag="py")
                       for j2 in range(NK)]
                pend = None
                for j in range(NJ):
                    c, jc = j // JPW, j % JPW
                    pa = pap.tile([128, QCH], F32, tag="pa")
                    for k in range(NK):
                        nc.tensor.matmul(
                            pa[:],
                            win_c[c][:, k, 128 * jc : 128 * (jc + 1)],
                            xts_q[q][:, k, :],
                            start=(k == 0),
                            stop=(k == NK - 1),
                        )
                    pb = pbp.tile([128, QCH], F32, tag="pb")
                    nc.tensor.matmul(
                        pb[:],
                        ab[:, 128 * j : 128 * (j + 1)],
                        cs[:, lo:hi],
                        start=True,
                        stop=True,
                    )
                    msb = modsp.tile([128, QCH], F32, tag="mods")
                    nc.scalar.copy(msb[:], pb[:])
                    hm = hmp.tile([128, QCH], BF16, tag="hm")
                    nc.vector.tensor_mul(hm[:], pa[:], msb[:])
                    if pend is not None:
                        phm, pj = pend
                        for j2 in range(NK):
                            nc.tensor.matmul(
                                pys[j2][:],
                                wout_g[pj // JPW][:, pj % JPW,
                                                  128 * j2 : 128 * (j2 + 1)],
                                phm[:],
                                start=(pj == 0),
                                stop=False,
                            )
                    pend = (hm, j)
                phm, pj = pend
                for j2 in range(NK):
                    nc.tensor.matmul(
                        pys[j2][:],
                        wout_g[pj // JPW][:, pj % JPW, 128 * j2 : 128 * (j2 + 1)],
                        phm[:],
                        start=(pj == 0),
                        stop=True,
                    )
                for j2 in range(NK):
                    yo = yop.tile([128, QCH], BF16, tag="yo")
                    if j2 % 2 == 0:
                        nc.scalar.copy(yo[:], pys[j2][:])
                        nc.scalar.dma_start(
                            yT_d[128 * j2 : 128 * (j2 + 1), lo:hi], yo[:]
                        )
                    else:
                        nc.vector.tensor_copy(yo[:], pys[j2][:])
                        nc.sync.dma_start(
                            yT_d[128 * j2 : 128 * (j2 + 1), lo:hi], yo[:]
                        )

    nc.finalize()
    return nc


def _get_nc():
    if "nc" not in _cache:
        _cache["nc"] = _build()
    return _cache["nc"]


def _bf16(a):
    import ml_dtypes
    return np.ascontiguousarray(np.asarray(a, dtype=np.float32).astype(ml_dtypes.bfloat16))


def _in_maps(x, input_proj, output_proj, floquet_energies, drive_weights,
             coupling_matrix):
    coupled = coupling_matrix.astype(np.float64) @ drive_weights.astype(np.float64)
    E = floquet_energies.astype(np.float64)
    ab_np = np.concatenate(
        [coupled[:, None] * np.cos(E), -coupled[:, None] * np.sin(E)], axis=0
    )
    ab = _bf16(ab_np)
    w_in = _bf16(input_proj)
    w_out = _bf16(output_proj)

    harm = np.arange(1, M + 1, dtype=np.float64)
    maps = []
    for c in range(NCORES):
        b, half = c // 2, c % 2
        t = (half * T + np.arange(T, dtype=np.float64)) / L
        ang = 2.0 * np.pi * harm[:, None] * t[None, :]
        cs = _bf16(np.concatenate([np.cos(ang), np.sin(ang)], axis=0))
        xT = _bf16(x[b, half * T : (half + 1) * T, :].T)
        maps.append(
            {
                "xT": xT,
                "w_in": w_in,
                "w_out": w_out,
                "cs": cs,
                "ab": ab,
            }
        )
    return maps


def kernel(x, input_proj, output_proj, floquet_energies, drive_weights,
           coupling_matrix, _trace=False, _trace_kwargs=None):
    from concourse.bass_utils import run_bass_kernel_spmd

    nc = _get_nc()
    maps = _in_maps(x, input_proj, output_proj, floquet_energies,
                    drive_weights, coupling_matrix)
    kw = dict(_trace_kwargs or {})
    res = run_bass_kernel_spmd(nc, maps, list(range(NCORES)), trace=_trace, **kw)
    out = np.empty((B, L, D), dtype=np.float32)
    for c in range(NCORES):
        b, half = c // 2, c % 2
        out[b, half * T : (half + 1) * T, :] = (
            res.results[c]["yT"].astype(np.float32).T
        )
    if _trace:
        return out, res
    return out


# revision 11
# speedup vs baseline: 1.1033x; 1.0937x over previous
"""HDTimeCrystalBlock kernel for 8 Trainium2 NeuronCores.

Math: out = ((x @ W_in) * mod[None]) @ W_out, where
  mod[l,h] = sum_m coupled[m] * cos(omega*(m+1)*t[l] + E[m,h])
With cos(a+b) = cos(a)cos(b) - sin(a)sin(b), mod is a K=32 matmul:
  mod[h,l] = sum_r ab[r,h] * cs[r,l]
  ab rows 0:16  = coupled[m]*cos(E[m,h]),  rows 16:32 = -coupled[m]*sin(E[m,h])
  cs rows 0:16  = cos(omega*(m+1)*t[l]),   rows 16:32 = sin(...)
ab/cs are tiny and precomputed on the HOST (the old on-device prep chain
kept the PE HAM-throttled at 1.2 GHz for the first ~35us).  Both are
shipped with the 32 rows duplicated at partitions 32:64 so consecutive
j-tiles' mod matmuls run CONCURRENTLY in different 32-row strips of the
PE array (row tiling): a pair of K=32 matmuls costs ~one N=512 slot.

Sharding: data-parallel over the 8192 tokens (B*L), 1024 per core; weights
replicated. Activations stay transposed ([feature, token]); all matmul
operands bf16 (1 col/cycle @ 2.4 GHz). Output stored bf16.

Startup discipline:
  - PE warm-up matmuls on a memset scratch tile (no DMA dependency): PE
    busy from ~7us, HAM un-throttles by ~10.5us when real data lands.
  - DMA queue heads are exactly the first-needed tiles (win j0-1 on sync,
    xts0 on scalar, ab+cs on gpsimd); the remaining ~8.5MB of weight
    streams are gated behind a 1-element copy that depends on xts0, so
    they cannot steal HBM bandwidth from the critical path.
Steady-loop discipline:
  - The main loop runs in j-PAIRS; the mm2 stage is software-pipelined one
    pair (two j) behind, so the pb->msb->hm chain has ~4.5us of slack.
  - mm2 writes 8 half-bank PSUM tiles ([128,256] x 2 halves x 4 d-tiles =
    4 banks, pool bufs=8) so consecutive q's outputs double-buffer and the
    yo output copies (interleaved on Act/DVE during the next q) are never
    on the critical path.
"""
import math

import numpy as np

B, L, D, HD, M = 4, 2048, 512, 4096, 16
NCORES = 8
T = (B * L) // NCORES          # tokens per core
QCH = 512                      # l-chunk (PSUM bank width in fp32)
HCH = QCH // 2                 # mm2 half-tile width
NQ = T // QCH
NJ = HD // 128                 # h-tiles
NK = D // 128                  # d-tiles
K2 = 2 * M                     # mod-matmul contraction (32)
NWARM = 9                      # scratch warm-up matmuls (N=512)

# j-tile ranges per DMA chunk for w_in / w_out ([lo, hi) in j-tiles)
WIN_PARTS = [(0, 2), (2, 4), (4, 8), (8, 12), (12, 16), (16, 20),
             (20, 24), (24, 28), (28, 32)]
WOUT_PARTS = [(0, 4), (4, 8), (8, 12), (12, 16), (16, 20),
              (20, 24), (24, 28), (28, 32)]

_cache = {}


def _build():
    from concourse import bacc, bass, mybir, tile

    F32 = mybir.dt.float32
    BF16 = mybir.dt.bfloat16
    PSUM = bass.MemorySpace.PSUM

    nc = bacc.Bacc("TRN2", target_bir_lowering=False, debug=False)

    xT_d = nc.dram_tensor("xT", [D, T], BF16, kind="ExternalInput")
    w_in_d = nc.dram_tensor("w_in", [D, HD], BF16, kind="ExternalInput")
    w_out_d = nc.dram_tensor("w_out", [HD, D], BF16, kind="ExternalInput")
    cs_d = nc.dram_tensor("cs", [2 * K2, T], BF16, kind="ExternalInput")
    ab_d = nc.dram_tensor("ab", [2 * K2, HD], BF16, kind="ExternalInput")
    yT_d = nc.dram_tensor("yT", [D, T], BF16, kind="ExternalOutput")

    with tile.TileContext(nc) as tc:
        with (
            tc.tile_pool(name="win", bufs=1) as winp,
            tc.tile_pool(name="wout", bufs=1) as woutp,
            tc.tile_pool(name="xts", bufs=1) as xtp,
            tc.tile_pool(name="small", bufs=1) as smallp,
            tc.tile_pool(name="hm", bufs=4) as hmp,
            tc.tile_pool(name="mods", bufs=4) as modsp,
            tc.tile_pool(name="yo", bufs=4) as yop,
            tc.tile_pool(name="pa", bufs=2, space=PSUM) as pap,
            tc.tile_pool(name="pb", bufs=2, space=PSUM) as pbp,
            tc.tile_pool(name="py", bufs=4, space=PSUM) as pyp,
        ):
            # ---- scratch warm-up tile, memset first thing on gpsimd ----
            wmt = smallp.tile([128, QCH], BF16, tag="wmt")
            nc.gpsimd.memset(wmt[:], 0.5)

            # ---- DMA schedule ----
            w_in_r = w_in_d.ap().rearrange("(k p) h -> p k h", p=128)
            w_out_r = w_out_d.ap().rearrange("(j p) i -> p j i", p=128)
            xT_r = xT_d.ap().rearrange("(k p) (q t) -> q p k t", p=128, q=NQ)

            win_c = [None] * len(WIN_PARTS)
            wout_g = [None] * len(WOUT_PARTS)
            xts_q = [None] * NQ

            def load_win(i):
                a, b = WIN_PARTS[i]
                t_ = winp.tile([128, NK, 128 * (b - a)], BF16,
                               name=f"win{i}", tag=f"win{i}")
                nc.sync.dma_start(t_[:], w_in_r[:, :, 128 * a : 128 * b])
                win_c[i] = t_

            def load_wout(i):
                a, b = WOUT_PARTS[i]
                tw = woutp.tile([128, b - a, D], BF16,
                                name=f"wout{i}", tag=f"wout{i}")
                nc.gpsimd.dma_start(tw[:], w_out_r[:, a:b, :])
                wout_g[i] = tw

            def load_xts(q, eng):
                tx = xtp.tile([128, NK, T // NQ], BF16, name=f"xts{q}", tag=f"xts{q}")
                eng.dma_start(tx[:], xT_r[q])
                xts_q[q] = tx

            ab = smallp.tile([2 * K2, HD], BF16, tag="ab")
            cs = smallp.tile([2 * K2, T], BF16, tag="cs")
            gate = smallp.tile([1, 1], BF16, tag="gate")

            load_xts(0, nc.scalar)        # scalar queue head
            load_win(0)                   # sync queue head: j0-1
            nc.gpsimd.dma_start(ab[:], ab_d[:])
            nc.gpsimd.dma_start(cs[:], cs_d[:])
            load_win(1)                   # j2-3 (sync, behind j0-1)
            # gate: everything below waits for xts0 before transferring
            nc.gpsimd.tensor_copy(gate[:], xts_q[0][0:1, 0, 0:1])
            load_wout(0)
            for i in range(2, len(WIN_PARTS)):
                load_win(i)
            for i in range(1, len(WOUT_PARTS)):
                load_wout(i)
            load_xts(1, nc.scalar)

            def win_slice(j, k):
                for i, (a, b) in enumerate(WIN_PARTS):
                    if a <= j < b:
                        return win_c[i][:, k, 128 * (j - a) : 128 * (j - a + 1)]
                raise AssertionError

            def wout_slice(j, j2):
                for i, (a, b) in enumerate(WOUT_PARTS):
                    if a <= j < b:
                        return wout_g[i][:, j - a, 128 * j2 : 128 * (j2 + 1)]
                raise AssertionError

            # ---- PE warm-up on scratch (HAM to K=8/8 by ~10.5us) ----
            for w in range(NWARM):
                pw = pap.tile([128, QCH], F32, name=f"warm{w}", tag="pa")
                nc.tensor.matmul(pw[:], wmt[:, 0:128], wmt[:], start=True, stop=True)

            def emit_yo_batch(pq, ppys):
                # 2 copies on Act, 2 on DVE, stores on the idle sync queue
                yos = []
                for j2 in range(NK):
                    yo = yop.tile([128, QCH], BF16, name=f"yo{pq}_{j2}", tag="yo")
                    if j2 % 2 == 0:
                        nc.scalar.copy(yo[:], ppys[j2][:])
                    else:
                        nc.vector.tensor_copy(yo[:], ppys[j2][:])
                    yos.append(yo)
                for j2 in range(NK):
                    nc.sync.dma_start(
                        yT_d[128 * j2 : 128 * (j2 + 1),
                             pq * QCH : (pq + 1) * QCH],
                        yos[j2][:],
                    )

            # ---- fused main loop: j-pairs, mm2 pipelined one pair behind ----
            prev_q = None  # (q, pys) drained at pair 1 of the next q
            for q in range(NQ):
                lo, hi = q * QCH, (q + 1) * QCH
                pys = [pyp.tile([128, QCH], F32, name=f"py{q}_{j2}", tag="py")
                       for j2 in range(NK)]
                pend = None
                for p in range(NJ // 2):
                    j0, j1 = 2 * p, 2 * p + 1
                    pa0 = pap.tile([128, QCH], F32, tag="pa")
                    for k in range(NK):
                        nc.tensor.matmul(pa0[:], win_slice(j0, k),
                                         xts_q[q][:, k, :],
                                         start=(k == 0), stop=(k == NK - 1))
                    pb0 = pbp.tile([128, QCH], F32, tag="pb")
                    nc.tensor.matmul(pb0[:], ab[0:K2, 128 * j0 : 128 * (j0 + 1)],
                                     cs[0:K2, lo:hi], start=True, stop=True)
                    pb1 = pbp.tile([128, QCH], F32, tag="pb")
                    nc.tensor.matmul(pb1[:],
                                     ab[K2 : 2 * K2, 128 * j1 : 128 * (j1 + 1)],
                                     cs[K2 : 2 * K2, lo:hi],
                                     start=True, stop=True)
                    msb0 = modsp.tile([128, QCH], F32, tag="mods")
                    nc.scalar.copy(msb0[:], pb0[:])
                    hm0 = hmp.tile([128, QCH], BF16, tag="hm")
                    nc.vector.tensor_mul(hm0[:], pa0[:], msb0[:])
                    pa1 = pap.tile([128, QCH], F32, tag="pa")
                    for k in range(NK):
                        nc.tensor.matmul(pa1[:], win_slice(j1, k),
                                         xts_q[q][:, k, :],
                                         start=(k == 0), stop=(k == NK - 1))
                    msb1 = modsp.tile([128, QCH], F32, tag="mods")
                    nc.scalar.copy(msb1[:], pb1[:])
                    hm1 = hmp.tile([128, QCH], BF16, tag="hm")
                    nc.vector.tensor_mul(hm1[:], pa1[:], msb1[:])
                    # previous q's outputs drain here (after pair 0's msb/mul
                    # so the pa/pb recycle chain is never behind the copies)
                    if p == 1 and prev_q is not None:
                        pq, ppys = prev_q
                        emit_yo_batch(pq, ppys)
                        prev_q = None
                    if pend is not None:
                        for (pj, phm) in pend:
                            for j2 in range(NK):
                                nc.tensor.matmul(
                                    pys[j2][:],
                                    wout_slice(pj, j2),
                                    phm[:],
                                    start=(pj == 0),
                                    stop=(pj == NJ - 1),
                                )
                    pend = [(j0, hm0), (j1, hm1)]
                # flush last pair's mm2
                for (pj, phm) in pend:
                    for j2 in range(NK):
                        nc.tensor.matmul(
                            pys[j2][:],
                            wout_slice(pj, j2),
                            phm[:],
                            start=(pj == 0),
                            stop=(pj == NJ - 1),
                        )
                prev_q = (q, pys)

            # tail: drain the last q's outputs
            pq, ppys = prev_q
            emit_yo_batch(pq, ppys)

    nc.finalize()
    return nc


def _get_nc():
    if "nc" not in _cache:
        _cache["nc"] = _build()
    return _cache["nc"]


def _bf16(a):
    import ml_dtypes
    return np.ascontiguousarray(np.asarray(a, dtype=np.float32).astype(ml_dtypes.bfloat16))


def _in_maps(x, input_proj, output_proj, floquet_energies, drive_weights,
             coupling_matrix):
    coupled = coupling_matrix.astype(np.float64) @ drive_weights.astype(np.float64)
    E = floquet_energies.astype(np.float64)
    ab_half = np.concatenate(
        [coupled[:, None] * np.cos(E), -coupled[:, None] * np.sin(E)], axis=0
    )
    ab = _bf16(np.concatenate([ab_half, ab_half], axis=0))
    w_in = _bf16(input_proj)
    w_out = _bf16(output_proj)

    harm = np.arange(1, M + 1, dtype=np.float64)
    maps = []
    for c in range(NCORES):
        b, half = c // 2, c % 2
        t = (half * T + np.arange(T, dtype=np.float64)) / L
        ang = 2.0 * np.pi * harm[:, None] * t[None, :]
        cs_half = np.concatenate([np.cos(ang), np.sin(ang)], axis=0)
        cs = _bf16(np.concatenate([cs_half, cs_half], axis=0))
        xT = _bf16(x[b, half * T : (half + 1) * T, :].T)
        maps.append(
            {
                "xT": xT,
                "w_in": w_in,
                "w_out": w_out,
                "cs": cs,
                "ab": ab,
            }
        )
    return maps


def kernel(x, input_proj, output_proj, floquet_energies, drive_weights,
           coupling_matrix, _trace=False, _trace_kwargs=None):
    from concourse.bass_utils import run_bass_kernel_spmd

    nc = _get_nc()
    maps = _in_maps(x, input_proj, output_proj, floquet_energies,
                    drive_weights, coupling_matrix)
    kw = dict(_trace_kwargs or {})
    res = run_bass_kernel_spmd(nc, maps, list(range(NCORES)), trace=_trace, **kw)
    out = np.empty((B, L, D), dtype=np.float32)
    for c in range(NCORES):
        b, half = c // 2, c % 2
        out[b, half * T : (half + 1) * T, :] = (
            res.results[c]["yT"].astype(np.float32).T
        )
    if _trace:
        return out, res
    return out
